# revision 44
# baseline (speedup 1.0000x reference)
"""Trainium2 Bass kernel for the masked-attention block (nn_MAB_61607010894006).

Sharding: data-parallel over batch B=8 across 8 NeuronCores (one batch row
per core, weights replicated, no collectives).

v2 design (vs the 249us v1 baseline):
  - bf16 activations+weights (fp32 PSUM accumulation, fp32 LN stats, fp32
    output). Halves DMA, enables FWL on LDWEIGHTS and DVE 2x perf modes.
  - Scores: two 512-wide matmuls per (head, k-tile) into 1-bank PSUM
    tiles; head PAIRS run concurrently on the PE via row tiling
    (contraction=64: heads 2t/2t+1 at array rows 0-63/64-127).
  - Masking without exp-bias: masked k tokens compacted out on the host;
    padded k columns are zero (scores=0, exp=1) and both the v rows and
    the denominator indicator row are 0 there, so pads contribute nothing
    to numerator or denominator.
  - Softmax denominator rides the po matmul as a 65th v row (indicator).
    ALL reciprocals/rsqrts are computed as exp(-ln(x)) / exp(-0.5 ln(x))
    on ScalarE: Ln and Exp share one ACT table set, so the kernel does a
    single ACT_TABLE_LOAD (v1 lost 26us to exp<->recip table thrash).
    (Custom DVE ops and gpsimd compute ops fail codegen in this
    toolchain - only gpsimd DMA works.)
  - Row->partitions broadcasts via tiny PE ones-matmuls; elementwise on
    DVE in bf16 (2x mode); odd-head partition shifts via gpsimd DMA.
"""

import sys

sys.path.insert(0, "/opt/trn_rl_repo")

import numpy as np
import ml_dtypes

import concourse.bass as bass
import concourse.mybir as mybir
import concourse.tile as tile
from concourse.bass_utils import run_bass_kernel_spmd


F32 = mybir.dt.float32
F32R = mybir.dt.float32r
BF16 = mybir.dt.bfloat16
AF = mybir.ActivationFunctionType

B, NQ, NK, D, H, DH = 8, 1024, 1024, 512, 8, 64
EPS = 1e-5
N_CORES = 8


def _split_multi_waits(nc):
    """This toolchain's walrus allows ONE sem wait per TPB instruction; Tile
    can emit several (kernel-tail drain). Hoist extras onto preceding
    single-wait NOPs on the same engine stream (equivalent: in-order issue).
    """
    multi_update = []
    for fn in nc.m.functions:
        for bb in fn.blocks:
            insts = bb.instructions
            new = []
            changed = False
            for inst in insts:
                si = inst.sync_info
                if si is not None and si.on_wait and len(si.on_wait) > 1:
                    waits = list(si.on_wait)
                    for w in waits[:-1]:
                        nop = mybir.InstNoOp(
                            name=f"I-wsplit-{nc.next_id()}", engine=inst.engine
                        )
                        nop.sync_info = mybir.SyncInfo(on_wait=[w], on_update=[])
                        new.append(nop)
                    inst.sync_info = mybir.SyncInfo(
                        on_wait=[waits[-1]], on_update=list(si.on_update)
                    )
                    changed = True
                if si is not None and si.on_update and len(si.on_update) > 1:
                    multi_update.append(inst.name)
                new.append(inst)
            if changed:
                bb.instructions = new
    if multi_update:
        raise RuntimeError(f">1 sem update unsupported: {multi_update[:10]}")


def build_nc(kt_tiles=5, affine=False):
    NKP = kt_tiles * 128  # compacted+padded key/value token count
    nc = bass.Bass()

    qt_d = nc.dram_tensor("qt", [D, NQ], BF16, kind="ExternalInput")
    kt_d = nc.dram_tensor("kt", [D + 1, NKP], BF16, kind="ExternalInput")  # +ind
    wq_d = nc.dram_tensor("wq", [D, D], BF16, kind="ExternalInput")
    wk_d = nc.dram_tensor("wk", [D, D], BF16, kind="ExternalInput")
    wv_d = nc.dram_tensor("wv", [D + 1, D], BF16, kind="ExternalInput")  # +bv row
    wo_d = nc.dram_tensor("wo", [D, D], BF16, kind="ExternalInput")
    bq_d = nc.dram_tensor("bq", [128, 4], F32, kind="ExternalInput")
    bk_d = nc.dram_tensor("bk", [128, 4], F32, kind="ExternalInput")
    bo_d = nc.dram_tensor("bo", [128, 4], F32, kind="ExternalInput")
    ind_d = nc.dram_tensor("ind", [128, kt_tiles], BF16, kind="ExternalInput")
    cr_d = nc.dram_tensor("cr", [2, 128], BF16, kind="ExternalInput")  # row0=ones
    gb_d = nc.dram_tensor("gb", [128, 16], F32, kind="ExternalInput")  # percol
    cn_d = nc.dram_tensor("cn", [128, 1], BF16, kind="ExternalInput")  # 1/512
    out_d = nc.dram_tensor("out", [D, NQ], BF16, kind="ExternalOutput")

    mult, add = mybir.AluOpType.mult, mybir.AluOpType.add

    def mm(out, lhsT, rhs, **kw):
        nc.tensor.matmul(out, lhsT, rhs, **kw)

    with tile.TileContext(nc) as tc:
        with (
            tc.tile_pool(name="wp", bufs=1) as wp,
            tc.tile_pool(name="ap", bufs=1) as ap,
            tc.tile_pool(name="sm", bufs=2) as sm,
            tc.tile_pool(name="pp", bufs=1, space="PSUM") as pp,
        ):
            def ps_tile(shape, name, tag, bufs):
                return pp.tile(shape, F32, name=name, tag=tag, bufs=bufs)

            # ---- small constants first (cheap DMAs) ---------------------------
            bq_sb = wp.tile([128, 4], F32, name="bq_sb")
            bk_sb = wp.tile([128, 4], F32, name="bk_sb")
            bo_sb = wp.tile([128, 4], F32, name="bo_sb")
            ind_sb = wp.tile([128, kt_tiles], BF16, name="ind_sb")
            cr_sb = wp.tile([2, 128], BF16, name="cr_sb")
            gb_sb = wp.tile([128, 16], F32, name="gb_sb")
            cn_sb = wp.tile([128, 1], BF16, name="cn_sb")
            nc.sync.dma_start(bq_sb[:], bq_d[:])
            nc.sync.dma_start(bk_sb[:], bk_d[:])
            nc.sync.dma_start(bo_sb[:], bo_d[:])
            nc.sync.dma_start(ind_sb[:], ind_d[:])
            nc.sync.dma_start(cr_sb[:], cr_d[:])
            nc.sync.dma_start(gb_sb[:], gb_d[:])
            nc.sync.dma_start(cn_sb[:], cn_d[:])
            eps_sb = wp.tile([1, 1], F32, name="eps_sb")
            nc.vector.memset(eps_sb[:], EPS)
            sum_lhs = cn_sb[:, 0:1]                    # [128,1] bf16 = 1/512
            ones_row = cr_sb[0:1, :]                   # [1,128] bf16 lhsT

            # ---- PE warmup while DMAs stream ---------------------------------
            wu_sb = wp.tile([128, 128], BF16, name="wu_sb")
            nc.vector.memset(wu_sb[:], 0.001)
            wu_ps = ps_tile([128, 512], "wu_ps", "s1", 2)
            for i in range(44):
                mm(wu_ps[:, 0:128], wu_sb[:], wu_sb[:],
                   start=(i == 0), stop=(i == 43))
            wu_out = wp.tile([1, 1], F32, name="wu_out")
            nc.vector.tensor_copy(wu_out[:], wu_ps[0:1, 0:1])

            # ---- weights ------------------------------------------------------
            wq_sb = wp.tile([128, 4 * D], BF16, name="wq_sb")
            wk_sb = wp.tile([128, 4 * D], BF16, name="wk_sb")
            wv_sb = wp.tile([128, 4 * D], BF16, name="wv_sb")
            wv1_sb = wp.tile([1, D], BF16, name="wv1_sb")
            wo_sb = wp.tile([128, 4 * D], BF16, name="wo_sb")
            for t in range(4):
                nc.scalar.dma_start(
                    wv_sb[:, t * D : (t + 1) * D], wv_d[t * 128 : (t + 1) * 128, :]
                )
            nc.scalar.dma_start(wv1_sb[:, :], wv_d[D : D + 1, :])

            # ---- staged inputs (kt first: v+k projections start earliest) ----
            kt_sb = wp.tile([128, 4 * NKP], BF16, name="kt_sb")
            kt1_sb = wp.tile([1, NKP], BF16, name="kt1_sb")
            for t in range(4):
                nc.sync.dma_start(
                    kt_sb[:, t * NKP : (t + 1) * NKP],
                    kt_d[t * 128 : (t + 1) * 128, :],
                )
            nc.sync.dma_start(kt1_sb[:, :], kt_d[D : D + 1, :])
            for t in range(4):
                nc.scalar.dma_start(
                    wk_sb[:, t * D : (t + 1) * D], wk_d[t * 128 : (t + 1) * 128, :]
                )
            qt_sb = wp.tile([128, 4 * NQ], BF16, name="qt_sb")
            for t in range(4):
                nc.sync.dma_start(
                    qt_sb[:, t * NQ : (t + 1) * NQ],
                    qt_d[t * 128 : (t + 1) * 128, :],
                )
            for t in range(4):
                nc.scalar.dma_start(
                    wq_sb[:, t * D : (t + 1) * D], wq_d[t * 128 : (t + 1) * 128, :]
                )
            for t in range(4):
                nc.sync.dma_start(
                    wo_sb[:, t * D : (t + 1) * D], wo_d[t * 128 : (t + 1) * 128, :]
                )

            # ---- persistent activations --------------------------------------
            q_bf = ap.tile([128, 4 * NQ], BF16, name="q_bf")
            k_bf = ap.tile([128, 4 * NKP], BF16, name="k_bf")
            v_sb = ap.tile([128, kt_tiles * (8 * 65)], BF16, name="v_sb")
            v_ones = v_sb.rearrange("p (v h x) -> p v h x", v=kt_tiles, h=8)[
                :, :, :, 64
            ]
            nc.vector.tensor_copy(
                v_ones,
                ind_sb.rearrange("p (v a) -> p v a", a=1)
                .broadcast_to([128, kt_tiles, 8]),
            )

            # ---- phase 1a: v projection (token-major, +bias via ind row) -----
            for vt in range(kt_tiles):
                ps_v = ps_tile([128, 512], "ps_v", "s1", 2)
                for kc in range(4):
                    mm(
                        ps_v[:],
                        kt_sb[:, kc * NKP + vt * 128 : kc * NKP + (vt + 1) * 128],
                        wv_sb[:, kc * D : (kc + 1) * D],
                        start=(kc == 0),
                        stop=False,
                    )
                mm(
                    ps_v[:],
                    kt1_sb[0:1, vt * 128 : (vt + 1) * 128],
                    wv1_sb[0:1, :],
                    start=False,
                    stop=True,
                )
                v_dst = v_sb[:, vt * 520 : (vt + 1) * 520].rearrange(
                    "p (h x) -> p h x", h=8
                )[:, :, 0:64]
                nc.scalar.copy(v_dst, ps_v.rearrange("p (h x) -> p h x", h=8))

            # ---- phases 1b+2 interleaved per t-block -------------------------
            o_bf = ap.tile([128, 4 * NQ], BF16, name="o_bf")
            sqt_tiles = []
            kchunks = [(0, 512), (512, NKP - 512)] if NKP > 512 else [(0, NKP)]

            def keepwarm(n, name):
                kw = ps_tile([128, 512], name, "po1", 3)
                for i in range(n):
                    mm(kw[:], wu_sb[:], wq_sb[:, 0:512],
                       start=(i == 0), stop=(i == n - 1))

            # ---- layernorm sums + stats + broadcast helpers -------------------
            def ln_sums_make(tag):
                return [ps_tile([65, 512], f"st{tag}{c}", "sx", 3)
                        for c in range(2)]

            def ln_sums_step(sts, x_bf, sq_tiles, ts):
                for c in range(2):
                    for t in ts:
                        csl = slice(t * NQ + c * 512, t * NQ + c * 512 + 512)
                        mm(sts[c][0:1, :], sum_lhs, x_bf[:, csl],
                           start=(t == 0), stop=(t == 3))
                        mm(sts[c][64:65, :], sum_lhs,
                           sq_tiles[t][:, c * 512 : c * 512 + 512],
                           start=(t == 0), stop=(t == 3))

            def ln_sums(x_bf, sq_tiles, tag):
                sts = ln_sums_make(tag)
                ln_sums_step(sts, x_bf, sq_tiles, range(4))
                return sts

            def ln_stats_and_reps(sts, tag):
                """sts: per-c [65,512] f32 PSUM (mean row 0, meansq row 64).
                Returns (rr, rm): [128, NQ] bf16 broadcasts of rstd, mu*rstd."""
                var = sm.tile([1, NQ], F32, name=f"var{tag}", tag="var", bufs=2)
                for c in range(2):
                    csl = slice(c * 512, (c + 1) * 512)
                    nc.scalar.activation(var[:, csl], sts[c][0:1, :], AF.Square)
                    nc.vector.tensor_sub(var[:, csl], sts[c][64:65, :],
                                         var[:, csl])
                # rstd = exp(-0.5*ln(var+eps)) — stays in the Exp table set;
                # per-c chains so chunk 0's broadcasts start before chunk 1
                rstd = sm.tile([1, NQ], BF16, name=f"rs{tag}", tag="rs", bufs=2)
                murm = sm.tile([1, NQ], BF16, name=f"mm{tag}", tag="mm2", bufs=2)
                rr = sm.tile([128, NQ], BF16, name=f"rrb{tag}", tag="rrb", bufs=2)
                rm = sm.tile([128, NQ], BF16, name=f"rmb{tag}", tag="rmb", bufs=2)
                for c in range(2):
                    csl = slice(c * 512, (c + 1) * 512)
                    nc.scalar.activation(var[:, csl], var[:, csl], AF.Ln,
                                         bias=eps_sb[0:1, 0:1])
                    nc.scalar.activation(rstd[:, csl], var[:, csl], AF.Exp,
                                         scale=-0.5)
                    nc.vector.tensor_mul(murm[:, csl], sts[c][0:1, :],
                                         rstd[:, csl])
                    rr_ps = ps_tile([128, 512], f"rr{tag}{c}", "sx", 3)
                    mm(rr_ps[:], ones_row, rstd[:, csl], start=True, stop=True)
                    nc.vector.tensor_copy(rr[:, csl], rr_ps[:])
                    rm_ps = ps_tile([128, 512], f"rm{tag}{c}", "sx", 3)
                    mm(rm_ps[:], ones_row, murm[:, csl], start=True, stop=True)
                    nc.vector.tensor_copy(rm[:, csl], rm_ps[:])
                return rr, rm


            for t in range(4):
                tsl = slice(t * NQ, (t + 1) * NQ)
                # -- k projection block t (kc-outer, weights reused) --
                ps_k = [
                    ps_tile([128, cw], f"ps_k{t}_{ci}", "s1", 2)
                    for ci, (cs, cw) in enumerate(kchunks)
                ]
                for kc in range(4):
                    for ci, (cs, cw) in enumerate(kchunks):
                        mm(
                            ps_k[ci][:],
                            wk_sb[:, kc * D + t * 128 : kc * D + (t + 1) * 128],
                            kt_sb[:, kc * NKP + cs : kc * NKP + cs + cw],
                            start=(kc == 0),
                            stop=(kc == 3),
                        )
                for ci, (cs, cw) in enumerate(kchunks):
                    nc.vector.tensor_scalar_add(
                        k_bf[:, t * NKP + cs : t * NKP + cs + cw],
                        ps_k[ci][:],
                        bk_sb[:, t : t + 1],
                    )
                # -- q projection block t --
                ps_q = [ps_tile([128, 512], f"ps_q{t}_{c}", "s1", 2) for c in range(2)]
                for kc in range(4):
                    for c in range(2):
                        mm(
                            ps_q[c][:],
                            wq_sb[:, kc * D + t * 128 : kc * D + (t + 1) * 128],
                            qt_sb[:, kc * NQ + c * 512 : kc * NQ + c * 512 + 512],
                            start=(kc == 0),
                            stop=(kc == 3),
                        )
                for c in range(2):
                    nc.vector.tensor_scalar_add(
                        q_bf[:, t * NQ + c * 512 : t * NQ + c * 512 + 512],
                        ps_q[c][:],
                        bq_sb[:, t : t + 1],
                    )

                # -- scores + exp for head pair (2t, 2t+1), row-tiled --
                at_tiles = {0: [], 1: []}
                for i in range(kt_tiles):
                    for sub in range(2):
                        rh = sub * 64
                        at_sb = ap.tile(
                            [128, NQ], BF16, name=f"at{t}_{i}_{sub}", tag="at",
                            bufs=16,
                        )
                        at_tiles[sub].append(at_sb)
                        for c in range(2):
                            ps_s = ps_tile([128, 512], f"s{t}_{i}_{sub}_{c}", "sx", 3)
                            mm(
                                ps_s[:],
                                k_bf[rh : rh + 64,
                                     t * NKP + i * 128 : t * NKP + (i + 1) * 128],
                                q_bf[rh : rh + 64,
                                     t * NQ + c * 512 : t * NQ + c * 512 + 512],
                                start=True,
                                stop=True,
                            )
                            nc.scalar.activation(
                                at_sb[:, c * 512 : (c + 1) * 512], ps_s[:],
                                AF.Exp, scale=0.125,
                            )

                # -- early LN0 sums (t<=2 blocks) during the last pair --
                if t == 3:
                    sts0 = ln_sums_make("l0")
                    ln_sums_step(sts0, o_bf, sqt_tiles, range(3))

                # -- A@V (both chunks), then normalize + residual --
                for sub in range(2):
                    h = 2 * t + sub
                    rh = sub * 64
                    pos = []
                    for c in range(2):
                        po = ps_tile([65, 512], f"po{h}_{c}", "po1", 3)
                        pos.append(po)
                        for i in range(kt_tiles):
                            mm(
                                po[:],
                                v_sb[:, i * 520 + h * 65 : i * 520 + (h + 1) * 65],
                                at_tiles[sub][i][:, c * 512 : (c + 1) * 512],
                                start=(i == 0),
                                stop=(i == kt_tiles - 1),
                            )
                    for c in range(2):
                        po = pos[c]
                        lnr = sm.tile([1, 512], F32, name=f"ln{h}{c}",
                                      tag="ri", bufs=2)
                        nc.scalar.activation(lnr[:], po[64:65, :], AF.Ln)
                        rinv = sm.tile([1, 512], BF16, name=f"r{h}{c}",
                                       tag="ri2", bufs=2)
                        nc.scalar.activation(rinv[:], lnr[:], AF.Exp,
                                             scale=-1.0)
                        pb = ps_tile([64, 512], f"pb{h}{c}", "po1", 3)
                        mm(pb[:], cr_sb[0:1, 0:64], rinv[:],
                           start=True, stop=True)
                        rb = sm.tile([64, 512], BF16, name=f"rb{h}{c}",
                                     tag="rb", bufs=2)
                        nc.vector.tensor_copy(rb[:], pb[:])
                        avn = sm.tile([64, 512], BF16, name=f"av{h}{c}",
                                      tag="av", bufs=2)
                        nc.vector.tensor_mul(avn[:], po[0:64, :], rb[:])
                        csl = slice(t * NQ + c * 512, t * NQ + c * 512 + 512)
                        if rh == 0:
                            nc.vector.tensor_add(
                                o_bf[0:64, csl], avn[:], q_bf[0:64, csl]
                            )
                        else:
                            av2 = sm.tile([128, 512], BF16, name=f"av2_{h}{c}",
                                          tag="av2", bufs=2)
                            nc.gpsimd.dma_start(av2[64:128, :], avn[:])
                            nc.vector.tensor_add(
                                o_bf[64:128, csl], av2[64:128, :],
                                q_bf[64:128, csl],
                            )

                # square of o block t for LN0 sumsq (sums deferred)
                sqt = sm.tile([128, NQ], BF16, name=f"sqt{t}", tag="sqt", bufs=4)
                sqt_tiles.append(sqt)
                nc.vector.tensor_mul(sqt[:], o_bf[:, tsl], o_bf[:, tsl])

            # ---- phase 3: LN0 -------------------------------------------------
            ot0 = ap.tile([128, 4 * NQ], BF16, name="ot0")
            ln_sums_step(sts0, o_bf, sqt_tiles, [3])
            keepwarm(20, "kw0")
            rr0, rm0 = ln_stats_and_reps(sts0, "l0")
            for c in range(2):
                for t in range(4):
                    sl = slice(t * NQ + c * 512, t * NQ + c * 512 + 512)
                    rsl = slice(c * 512, c * 512 + 512)
                    nc.vector.tensor_mul(ot0[:, sl], o_bf[:, sl], rr0[:, rsl])
                    nc.vector.tensor_sub(ot0[:, sl], ot0[:, sl], rm0[:, rsl])
                    if affine:
                        nc.vector.tensor_scalar(
                            ot0[:, sl], ot0[:, sl],
                            gb_sb[:, 0 + t : 0 + t + 1],
                            gb_sb[:, 4 + t : 4 + t + 1],
                            mult, add,
                        )

            # ---- phase 4: FC + relu + residual, token-chunk split ------------
            o1 = ap.tile([128, 4 * NQ], BF16, name="o1")
            sq1_tiles = [
                sm.tile([128, NQ], BF16, name=f"sq1t{ot}", tag="sqt", bufs=4)
                for ot in range(4)
            ]
            sts1 = ln_sums_make("l1")
            for c in range(2):
                for ot in range(4):
                    ps_f = ps_tile([128, 512], f"psf{ot}_{c}", "s1", 2)
                    for ft in range(4):
                        mm(
                            ps_f[:],
                            wo_sb[:, ft * D + ot * 128 : ft * D + (ot + 1) * 128],
                            ot0[:, ft * NQ + c * 512 : ft * NQ + c * 512 + 512],
                            start=(ft == 0),
                            stop=(ft == 3),
                        )
                    csl = slice(ot * NQ + c * 512, ot * NQ + c * 512 + 512)
                    rl = sm.tile([128, 512], BF16, name=f"rl{ot}{c}", tag="rl",
                                 bufs=2)
                    nc.scalar.activation(
                        rl[:], ps_f[:], AF.Relu, bias=bo_sb[:, ot : ot + 1]
                    )
                    nc.vector.tensor_add(o1[:, csl], ot0[:, csl], rl[:])
                    sq1t = sq1_tiles[ot]
                    nc.scalar.activation(
                        sq1t[:, c * 512 : c * 512 + 512], o1[:, csl], AF.Square
                    )
                    mm(sts1[c][0:1, :], sum_lhs, o1[:, csl],
                       start=(ot == 0), stop=(ot == 3))
                    mm(sts1[c][64:65, :], sum_lhs,
                       sq1t[:, c * 512 : c * 512 + 512],
                       start=(ot == 0), stop=(ot == 3))

            # ---- phase 5: LN1 -> out ------------------------------------------
            otout = ap.tile([128, 4 * NQ], BF16, name="otout")
            rr1, rm1 = ln_stats_and_reps(sts1, "l1")
            for t in range(4):
                for c in range(2):
                    sl = slice(t * NQ + c * 512, t * NQ + c * 512 + 512)
                    rsl = slice(c * 512, c * 512 + 512)
                    tmp = sm.tile([128, 512], BF16, name=f"tmp{t}{c}", tag="rl",
                                  bufs=2)
                    nc.vector.tensor_mul(tmp[:], o1[:, sl], rr1[:, rsl])
                    if affine:
                        nc.vector.tensor_sub(tmp[:], tmp[:], rm1[:, rsl])
                        nc.vector.tensor_scalar(
                            otout[:, sl], tmp[:],
                            gb_sb[:, 8 + t : 8 + t + 1],
                            gb_sb[:, 12 + t : 12 + t + 1],
                            mult, add,
                        )
                    else:
                        nc.vector.tensor_sub(otout[:, sl], tmp[:], rm1[:, rsl])
                    nc.sync.dma_start(
                        out_d[t * 128 : (t + 1) * 128, c * 512 : c * 512 + 512],
                        otout[:, sl],
                    )

    _split_multi_waits(nc)
    return nc


_nc_cache = {}


def _get_nc(kt_tiles=5, affine=False):
    key = (kt_tiles, affine)
    if key not in _nc_cache:
        _nc_cache[key] = build_nc(kt_tiles, affine)
    return _nc_cache[key]


def _kt_tiles_for(mask):
    n = int(max(int((mask[b] != 0).sum()) for b in range(mask.shape[0])))
    return max(1, (n + 127) // 128)


def _is_affine(g0, b0, g1, b1):
    return not (
        np.all(np.asarray(g0) == 1.0)
        and np.all(np.asarray(b0) == 0.0)
        and np.all(np.asarray(g1) == 1.0)
        and np.all(np.asarray(b1) == 0.0)
    )


def prep_inputs(Q, K, mask, Wq, bq, Wk, bk, Wv, bv, Wo, bo, g0, b0, g1, b1,
                kt_tiles=None):
    f32, bf = np.float32, ml_dtypes.bfloat16
    if kt_tiles is None:
        kt_tiles = _kt_tiles_for(np.asarray(mask))
    nkp = kt_tiles * 128

    def percol(v):  # [512] feature vector -> [128, 4] per-partition layout
        return np.ascontiguousarray(np.asarray(v, f32).reshape(4, 128).T)

    wv_h = np.ascontiguousarray(
        np.vstack([np.asarray(Wv, f32), np.asarray(bv, f32)[None, :]])
    ).astype(bf)
    cr = np.zeros((2, 128), f32)
    cr[0, :] = 1.0
    cr = cr.astype(bf)
    gb = np.concatenate(
        [percol(g0), percol(b0), percol(g1), percol(b1)], axis=1
    ).astype(f32)
    cn = np.full((128, 1), 1.0 / D, f32).astype(bf)
    wq_h = np.ascontiguousarray(np.asarray(Wq, f32)).astype(bf)
    wk_h = np.ascontiguousarray(np.asarray(Wk, f32)).astype(bf)
    wo_h = np.ascontiguousarray(np.asarray(Wo, f32)).astype(bf)

    in_maps = []
    for b in range(B):
        qt = np.ascontiguousarray(np.asarray(Q[b], f32).T).astype(bf)
        idx = np.nonzero(np.asarray(mask)[b] != 0)[0]
        kc = np.zeros((nkp, D), f32)
        kc[: len(idx)] = np.asarray(K[b], f32)[idx]
        indrow = np.zeros((1, nkp), f32)
        indrow[0, : len(idx)] = 1.0
        kt = np.ascontiguousarray(np.vstack([kc.T, indrow])).astype(bf)
        ind = np.ascontiguousarray(indrow.reshape(kt_tiles, 128).T).astype(bf)
        in_maps.append(
            {
                "qt": qt,
                "kt": kt,
                "wq": wq_h,
                "wk": wk_h,
                "wv": wv_h,
                "wo": wo_h,
                "bq": percol(bq),
                "bk": percol(bk),
                "bo": percol(bo),
                "ind": ind,
                "cr": cr,
                "gb": gb,
                "cn": cn,
            }
        )
    return in_maps


def kernel(Q, K, mask, Wq, bq, Wk, bk, Wv, bv, Wo, bo, g0, b0, g1, b1):
    mask = np.asarray(mask)
    kt_tiles = _kt_tiles_for(mask)
    affine = _is_affine(g0, b0, g1, b1)
    nc = _get_nc(kt_tiles, affine)
    in_maps = prep_inputs(
        Q, K, mask, Wq, bq, Wk, bk, Wv, bv, Wo, bo, g0, b0, g1, b1, kt_tiles
    )
    res = run_bass_kernel_spmd(nc, in_maps, list(range(N_CORES)))
    out = np.stack(
        [np.ascontiguousarray(res.results[i]["out"].T) for i in range(N_CORES)]
    )
    return out.astype(np.float32)


# revision 45
# speedup vs baseline: 1.0073x; 1.0073x over previous
"""Trainium2 Bass kernel for the masked-attention block (nn_MAB_61607010894006).

Sharding: data-parallel over batch B=8 across 8 NeuronCores (one batch row
per core, weights replicated, no collectives).

v2 design (vs the 249us v1 baseline):
  - bf16 activations+weights (fp32 PSUM accumulation, fp32 LN stats, fp32
    output). Halves DMA, enables FWL on LDWEIGHTS and DVE 2x perf modes.
  - Scores: two 512-wide matmuls per (head, k-tile) into 1-bank PSUM
    tiles; head PAIRS run concurrently on the PE via row tiling
    (contraction=64: heads 2t/2t+1 at array rows 0-63/64-127).
  - Masking without exp-bias: masked k tokens compacted out on the host;
    padded k columns are zero (scores=0, exp=1) and both the v rows and
    the denominator indicator row are 0 there, so pads contribute nothing
    to numerator or denominator.
  - Softmax denominator rides the po matmul as a 65th v row (indicator).
    ALL reciprocals/rsqrts are computed as exp(-ln(x)) / exp(-0.5 ln(x))
    on ScalarE: Ln and Exp share one ACT table set, so the kernel does a
    single ACT_TABLE_LOAD (v1 lost 26us to exp<->recip table thrash).
    (Custom DVE ops and gpsimd compute ops fail codegen in this
    toolchain - only gpsimd DMA works.)
  - Row->partitions broadcasts via tiny PE ones-matmuls; elementwise on
    DVE in bf16 (2x mode); odd-head partition shifts via gpsimd DMA.
"""

import sys

sys.path.insert(0, "/opt/trn_rl_repo")

import numpy as np
import ml_dtypes

import concourse.bass as bass
import concourse.mybir as mybir
import concourse.tile as tile
from concourse.bass_utils import run_bass_kernel_spmd


F32 = mybir.dt.float32
F32R = mybir.dt.float32r
BF16 = mybir.dt.bfloat16
AF = mybir.ActivationFunctionType

B, NQ, NK, D, H, DH = 8, 1024, 1024, 512, 8, 64
EPS = 1e-5
N_CORES = 8


def _split_multi_waits(nc):
    """This toolchain's walrus allows ONE sem wait per TPB instruction; Tile
    can emit several (kernel-tail drain). Hoist extras onto preceding
    single-wait NOPs on the same engine stream (equivalent: in-order issue).
    """
    multi_update = []
    for fn in nc.m.functions:
        for bb in fn.blocks:
            insts = bb.instructions
            new = []
            changed = False
            for inst in insts:
                si = inst.sync_info
                if si is not None and si.on_wait and len(si.on_wait) > 1:
                    waits = list(si.on_wait)
                    for w in waits[:-1]:
                        nop = mybir.InstNoOp(
                            name=f"I-wsplit-{nc.next_id()}", engine=inst.engine
                        )
                        nop.sync_info = mybir.SyncInfo(on_wait=[w], on_update=[])
                        new.append(nop)
                    inst.sync_info = mybir.SyncInfo(
                        on_wait=[waits[-1]], on_update=list(si.on_update)
                    )
                    changed = True
                if si is not None and si.on_update and len(si.on_update) > 1:
                    multi_update.append(inst.name)
                new.append(inst)
            if changed:
                bb.instructions = new
    if multi_update:
        raise RuntimeError(f">1 sem update unsupported: {multi_update[:10]}")


def build_nc(kt_tiles=5, affine=False):
    NKP = kt_tiles * 128  # compacted+padded key/value token count
    nc = bass.Bass()

    qt_d = nc.dram_tensor("qt", [D, NQ], BF16, kind="ExternalInput")
    kt_d = nc.dram_tensor("kt", [D + 1, NKP], BF16, kind="ExternalInput")  # +ind
    wq_d = nc.dram_tensor("wq", [D, D], BF16, kind="ExternalInput")
    wk_d = nc.dram_tensor("wk", [D, D], BF16, kind="ExternalInput")
    wv_d = nc.dram_tensor("wv", [D + 1, D], BF16, kind="ExternalInput")  # +bv row
    wo_d = nc.dram_tensor("wo", [D, D], BF16, kind="ExternalInput")
    bq_d = nc.dram_tensor("bq", [128, 4], F32, kind="ExternalInput")
    bk_d = nc.dram_tensor("bk", [128, 4], F32, kind="ExternalInput")
    bo_d = nc.dram_tensor("bo", [128, 4], F32, kind="ExternalInput")
    ind_d = nc.dram_tensor("ind", [128, kt_tiles], BF16, kind="ExternalInput")
    cr_d = nc.dram_tensor("cr", [2, 128], BF16, kind="ExternalInput")  # row0=ones
    gb_d = nc.dram_tensor("gb", [128, 16], F32, kind="ExternalInput")  # percol
    cn_d = nc.dram_tensor("cn", [128, 1], BF16, kind="ExternalInput")  # 1/512
    out_d = nc.dram_tensor("out", [D, NQ], BF16, kind="ExternalOutput")

    mult, add = mybir.AluOpType.mult, mybir.AluOpType.add

    def mm(out, lhsT, rhs, **kw):
        nc.tensor.matmul(out, lhsT, rhs, **kw)

    with tile.TileContext(nc) as tc:
        with (
            tc.tile_pool(name="wp", bufs=1) as wp,
            tc.tile_pool(name="ap", bufs=1) as ap,
            tc.tile_pool(name="sm", bufs=2) as sm,
            tc.tile_pool(name="pp", bufs=1, space="PSUM") as pp,
        ):
            def ps_tile(shape, name, tag, bufs):
                return pp.tile(shape, F32, name=name, tag=tag, bufs=bufs)

            # ---- small constants first (cheap DMAs) ---------------------------
            bq_sb = wp.tile([128, 4], F32, name="bq_sb")
            bk_sb = wp.tile([128, 4], F32, name="bk_sb")
            bo_sb = wp.tile([128, 4], F32, name="bo_sb")
            ind_sb = wp.tile([128, kt_tiles], BF16, name="ind_sb")
            cr_sb = wp.tile([2, 128], BF16, name="cr_sb")
            gb_sb = wp.tile([128, 16], F32, name="gb_sb")
            cn_sb = wp.tile([128, 1], BF16, name="cn_sb")
            nc.sync.dma_start(bq_sb[:], bq_d[:])
            nc.sync.dma_start(bk_sb[:], bk_d[:])
            nc.sync.dma_start(bo_sb[:], bo_d[:])
            nc.sync.dma_start(ind_sb[:], ind_d[:])
            nc.sync.dma_start(cr_sb[:], cr_d[:])
            nc.sync.dma_start(gb_sb[:], gb_d[:])
            nc.sync.dma_start(cn_sb[:], cn_d[:])
            eps_sb = wp.tile([1, 1], F32, name="eps_sb")
            nc.vector.memset(eps_sb[:], EPS)
            sum_lhs = cn_sb[:, 0:1]                    # [128,1] bf16 = 1/512
            ones_row = cr_sb[0:1, :]                   # [1,128] bf16 lhsT

            # ---- PE warmup while DMAs stream ---------------------------------
            wu_sb = wp.tile([128, 128], BF16, name="wu_sb")
            nc.vector.memset(wu_sb[:], 0.001)
            wu_ps = ps_tile([128, 512], "wu_ps", "s1", 2)
            for i in range(44):
                mm(wu_ps[:, 0:128], wu_sb[:], wu_sb[:],
                   start=(i == 0), stop=(i == 43))
            wu_out = wp.tile([1, 1], F32, name="wu_out")
            nc.vector.tensor_copy(wu_out[:], wu_ps[0:1, 0:1])

            # ---- weights ------------------------------------------------------
            wq_sb = wp.tile([128, 4 * D], BF16, name="wq_sb")
            wk_sb = wp.tile([128, 4 * D], BF16, name="wk_sb")
            wv_sb = wp.tile([128, 4 * D], BF16, name="wv_sb")
            wv1_sb = wp.tile([1, D], BF16, name="wv1_sb")
            wo_sb = wp.tile([128, 4 * D], BF16, name="wo_sb")
            for t in range(4):
                nc.scalar.dma_start(
                    wv_sb[:, t * D : (t + 1) * D], wv_d[t * 128 : (t + 1) * 128, :]
                )
            nc.scalar.dma_start(wv1_sb[:, :], wv_d[D : D + 1, :])

            # ---- staged inputs (kt first: v+k projections start earliest) ----
            kt_sb = wp.tile([128, 4 * NKP], BF16, name="kt_sb")
            kt1_sb = wp.tile([1, NKP], BF16, name="kt1_sb")
            for t in range(4):
                nc.sync.dma_start(
                    kt_sb[:, t * NKP : (t + 1) * NKP],
                    kt_d[t * 128 : (t + 1) * 128, :],
                )
            nc.sync.dma_start(kt1_sb[:, :], kt_d[D : D + 1, :])
            for t in range(4):
                nc.scalar.dma_start(
                    wk_sb[:, t * D : (t + 1) * D], wk_d[t * 128 : (t + 1) * 128, :]
                )
            qt_sb = wp.tile([128, 4 * NQ], BF16, name="qt_sb")
            for t in range(4):
                nc.sync.dma_start(
                    qt_sb[:, t * NQ : (t + 1) * NQ],
                    qt_d[t * 128 : (t + 1) * 128, :],
                )
            for t in range(4):
                nc.scalar.dma_start(
                    wq_sb[:, t * D : (t + 1) * D], wq_d[t * 128 : (t + 1) * 128, :]
                )
            for t in range(4):
                nc.sync.dma_start(
                    wo_sb[:, t * D : (t + 1) * D], wo_d[t * 128 : (t + 1) * 128, :]
                )

            # ---- persistent activations --------------------------------------
            q_bf = ap.tile([128, 4 * NQ], BF16, name="q_bf")
            k_bf = ap.tile([128, 4 * NKP], BF16, name="k_bf")
            v_sb = ap.tile([128, kt_tiles * (8 * 65)], BF16, name="v_sb")
            v_ones = v_sb.rearrange("p (v h x) -> p v h x", v=kt_tiles, h=8)[
                :, :, :, 64
            ]
            nc.vector.tensor_copy(
                v_ones,
                ind_sb.rearrange("p (v a) -> p v a", a=1)
                .broadcast_to([128, kt_tiles, 8]),
            )

            # ---- phase 1a: v projection (token-major, +bias via ind row) -----
            for vt in range(kt_tiles):
                ps_v = ps_tile([128, 512], "ps_v", "s1", 2)
                for kc in range(4):
                    mm(
                        ps_v[:],
                        kt_sb[:, kc * NKP + vt * 128 : kc * NKP + (vt + 1) * 128],
                        wv_sb[:, kc * D : (kc + 1) * D],
                        start=(kc == 0),
                        stop=False,
                    )
                mm(
                    ps_v[:],
                    kt1_sb[0:1, vt * 128 : (vt + 1) * 128],
                    wv1_sb[0:1, :],
                    start=False,
                    stop=True,
                )
                v_dst = v_sb[:, vt * 520 : (vt + 1) * 520].rearrange(
                    "p (h x) -> p h x", h=8
                )[:, :, 0:64]
                nc.scalar.copy(v_dst, ps_v.rearrange("p (h x) -> p h x", h=8))

            # ---- phases 1b+2 interleaved per t-block -------------------------
            o_bf = ap.tile([128, 4 * NQ], BF16, name="o_bf")
            sqt_tiles = []
            kchunks = [(0, 512), (512, NKP - 512)] if NKP > 512 else [(0, NKP)]

            def keepwarm(n, name):
                kw = ps_tile([128, 512], name, "po1", 3)
                for i in range(n):
                    mm(kw[:], wu_sb[:], wq_sb[:, 0:512],
                       start=(i == 0), stop=(i == n - 1))

            # ---- layernorm sums + stats + broadcast helpers -------------------
            def ln_sums_make(tag):
                return [ps_tile([65, 512], f"st{tag}{c}", "sx", 3)
                        for c in range(2)]

            def ln_sums_step(sts, x_bf, sq_tiles, ts):
                for c in range(2):
                    for t in ts:
                        csl = slice(t * NQ + c * 512, t * NQ + c * 512 + 512)
                        mm(sts[c][0:1, :], sum_lhs, x_bf[:, csl],
                           start=(t == 0), stop=(t == 3))
                        mm(sts[c][64:65, :], sum_lhs,
                           sq_tiles[t][:, c * 512 : c * 512 + 512],
                           start=(t == 0), stop=(t == 3))

            def ln_sums(x_bf, sq_tiles, tag):
                sts = ln_sums_make(tag)
                ln_sums_step(sts, x_bf, sq_tiles, range(4))
                return sts

            def ln_stats_and_reps(sts, tag):
                """sts: per-c [65,512] f32 PSUM (mean row 0, meansq row 64).
                Returns (rr, rm): [128, NQ] bf16 broadcasts of rstd, mu*rstd."""
                var = sm.tile([1, NQ], F32, name=f"var{tag}", tag="var", bufs=2)
                for c in range(2):
                    csl = slice(c * 512, (c + 1) * 512)
                    nc.scalar.activation(var[:, csl], sts[c][0:1, :], AF.Square)
                    nc.vector.tensor_sub(var[:, csl], sts[c][64:65, :],
                                         var[:, csl])
                # rstd = exp(-0.5*ln(var+eps)) — stays in the Exp table set;
                # per-c chains so chunk 0's broadcasts start before chunk 1
                rstd = sm.tile([1, NQ], BF16, name=f"rs{tag}", tag="rs", bufs=2)
                murm = sm.tile([1, NQ], BF16, name=f"mm{tag}", tag="mm2", bufs=2)
                rr = sm.tile([128, NQ], BF16, name=f"rrb{tag}", tag="rrb", bufs=2)
                rm = sm.tile([128, NQ], BF16, name=f"rmb{tag}", tag="rmb", bufs=2)
                for c in range(2):
                    csl = slice(c * 512, (c + 1) * 512)
                    nc.scalar.activation(var[:, csl], var[:, csl], AF.Ln,
                                         bias=eps_sb[0:1, 0:1])
                    nc.scalar.activation(rstd[:, csl], var[:, csl], AF.Exp,
                                         scale=-0.5)
                    nc.vector.tensor_mul(murm[:, csl], sts[c][0:1, :],
                                         rstd[:, csl])
                    rr_ps = ps_tile([128, 512], f"rr{tag}{c}", "sx", 3)
                    mm(rr_ps[:], ones_row, rstd[:, csl], start=True, stop=True)
                    nc.vector.tensor_copy(rr[:, csl], rr_ps[:])
                    rm_ps = ps_tile([128, 512], f"rm{tag}{c}", "sx", 3)
                    mm(rm_ps[:], ones_row, murm[:, csl], start=True, stop=True)
                    nc.vector.tensor_copy(rm[:, csl], rm_ps[:])
                return rr, rm


            for t in range(4):
                tsl = slice(t * NQ, (t + 1) * NQ)
                # -- k projection block t (kc-outer, weights reused) --
                ps_k = [
                    ps_tile([128, cw], f"ps_k{t}_{ci}", "s1", 2)
                    for ci, (cs, cw) in enumerate(kchunks)
                ]
                for kc in range(4):
                    for ci, (cs, cw) in enumerate(kchunks):
                        mm(
                            ps_k[ci][:],
                            wk_sb[:, kc * D + t * 128 : kc * D + (t + 1) * 128],
                            kt_sb[:, kc * NKP + cs : kc * NKP + cs + cw],
                            start=(kc == 0),
                            stop=(kc == 3),
                        )
                for ci, (cs, cw) in enumerate(kchunks):
                    nc.vector.tensor_scalar_add(
                        k_bf[:, t * NKP + cs : t * NKP + cs + cw],
                        ps_k[ci][:],
                        bk_sb[:, t : t + 1],
                    )
                # -- q projection block t --
                ps_q = [ps_tile([128, 512], f"ps_q{t}_{c}", "s1", 2) for c in range(2)]
                for kc in range(4):
                    for c in range(2):
                        mm(
                            ps_q[c][:],
                            wq_sb[:, kc * D + t * 128 : kc * D + (t + 1) * 128],
                            qt_sb[:, kc * NQ + c * 512 : kc * NQ + c * 512 + 512],
                            start=(kc == 0),
                            stop=(kc == 3),
                        )
                for c in range(2):
                    nc.vector.tensor_scalar_add(
                        q_bf[:, t * NQ + c * 512 : t * NQ + c * 512 + 512],
                        ps_q[c][:],
                        bq_sb[:, t : t + 1],
                    )

                # -- scores + exp for head pair (2t, 2t+1), row-tiled --
                at_tiles = {0: [], 1: []}
                for i in range(kt_tiles):
                    for sub in range(2):
                        rh = sub * 64
                        at_sb = ap.tile(
                            [128, NQ], BF16, name=f"at{t}_{i}_{sub}", tag="at",
                            bufs=16,
                        )
                        at_tiles[sub].append(at_sb)
                        for c in range(2):
                            ps_s = ps_tile([128, 512], f"s{t}_{i}_{sub}_{c}", "sx", 3)
                            mm(
                                ps_s[:],
                                k_bf[rh : rh + 64,
                                     t * NKP + i * 128 : t * NKP + (i + 1) * 128],
                                q_bf[rh : rh + 64,
                                     t * NQ + c * 512 : t * NQ + c * 512 + 512],
                                start=True,
                                stop=True,
                            )
                            nc.scalar.activation(
                                at_sb[:, c * 512 : (c + 1) * 512], ps_s[:],
                                AF.Exp, scale=0.125,
                            )

                # -- early LN0 sums (t<=2 blocks) during the last pair --
                if t == 3:
                    sts0 = ln_sums_make("l0")
                    ln_sums_step(sts0, o_bf, sqt_tiles, range(3))

                # -- A@V (both chunks), then normalize + residual --
                for sub in range(2):
                    h = 2 * t + sub
                    rh = sub * 64
                    pos = []
                    for c in range(2):
                        po = ps_tile([65, 512], f"po{h}_{c}", "po1", 3)
                        pos.append(po)
                        for i in range(kt_tiles):
                            mm(
                                po[:],
                                v_sb[:, i * 520 + h * 65 : i * 520 + (h + 1) * 65],
                                at_tiles[sub][i][:, c * 512 : (c + 1) * 512],
                                start=(i == 0),
                                stop=(i == kt_tiles - 1),
                            )
                    for c in range(2):
                        po = pos[c]
                        lnr = sm.tile([1, 512], F32, name=f"ln{h}{c}",
                                      tag="ri", bufs=2)
                        nc.scalar.activation(lnr[:], po[64:65, :], AF.Ln)
                        rinv = sm.tile([1, 512], BF16, name=f"r{h}{c}",
                                       tag="ri2", bufs=2)
                        nc.scalar.activation(rinv[:], lnr[:], AF.Exp,
                                             scale=-1.0)
                        pb = ps_tile([64, 512], f"pb{h}{c}", "po1", 3)
                        mm(pb[:], cr_sb[0:1, 0:64], rinv[:],
                           start=True, stop=True)
                        rb = sm.tile([64, 512], BF16, name=f"rb{h}{c}",
                                     tag="rb", bufs=2)
                        nc.vector.tensor_copy(rb[:], pb[:])
                        avn = sm.tile([64, 512], BF16, name=f"av{h}{c}",
                                      tag="av", bufs=2)
                        nc.vector.tensor_mul(avn[:], po[0:64, :], rb[:])
                        csl = slice(t * NQ + c * 512, t * NQ + c * 512 + 512)
                        if rh == 0:
                            nc.vector.tensor_add(
                                o_bf[0:64, csl], avn[:], q_bf[0:64, csl]
                            )
                        else:
                            av2 = sm.tile([128, 512], BF16, name=f"av2_{h}{c}",
                                          tag="av2", bufs=2)
                            nc.gpsimd.dma_start(av2[64:128, :], avn[:])
                            nc.vector.tensor_add(
                                o_bf[64:128, csl], av2[64:128, :],
                                q_bf[64:128, csl],
                            )

                # square of o block t for LN0 sumsq (sums deferred)
                sqt = sm.tile([128, NQ], BF16, name=f"sqt{t}", tag="sqt", bufs=4)
                sqt_tiles.append(sqt)
                nc.vector.tensor_mul(sqt[:], o_bf[:, tsl], o_bf[:, tsl])

            # ---- phase 3: LN0 -------------------------------------------------
            ot0 = ap.tile([128, 4 * NQ], BF16, name="ot0")
            ln_sums_step(sts0, o_bf, sqt_tiles, [3])
            rr0, rm0 = ln_stats_and_reps(sts0, "l0")
            for c in range(2):
                for t in range(4):
                    sl = slice(t * NQ + c * 512, t * NQ + c * 512 + 512)
                    rsl = slice(c * 512, c * 512 + 512)
                    nc.vector.tensor_mul(ot0[:, sl], o_bf[:, sl], rr0[:, rsl])
                    nc.vector.tensor_sub(ot0[:, sl], ot0[:, sl], rm0[:, rsl])
                    if affine:
                        nc.vector.tensor_scalar(
                            ot0[:, sl], ot0[:, sl],
                            gb_sb[:, 0 + t : 0 + t + 1],
                            gb_sb[:, 4 + t : 4 + t + 1],
                            mult, add,
                        )

            # ---- phase 4: FC + relu + residual, token-chunk split ------------
            o1 = ap.tile([128, 4 * NQ], BF16, name="o1")
            sq1_tiles = [
                sm.tile([128, NQ], BF16, name=f"sq1t{ot}", tag="sqt", bufs=4)
                for ot in range(4)
            ]
            sts1 = ln_sums_make("l1")
            for c in range(2):
                for ot in range(4):
                    ps_f = ps_tile([128, 512], f"psf{ot}_{c}", "s1", 2)
                    for ft in range(4):
                        mm(
                            ps_f[:],
                            wo_sb[:, ft * D + ot * 128 : ft * D + (ot + 1) * 128],
                            ot0[:, ft * NQ + c * 512 : ft * NQ + c * 512 + 512],
                            start=(ft == 0),
                            stop=(ft == 3),
                        )
                    csl = slice(ot * NQ + c * 512, ot * NQ + c * 512 + 512)
                    rl = sm.tile([128, 512], BF16, name=f"rl{ot}{c}", tag="rl",
                                 bufs=2)
                    nc.scalar.activation(
                        rl[:], ps_f[:], AF.Relu, bias=bo_sb[:, ot : ot + 1]
                    )
                    nc.vector.tensor_add(o1[:, csl], ot0[:, csl], rl[:])
                    sq1t = sq1_tiles[ot]
                    nc.scalar.activation(
                        sq1t[:, c * 512 : c * 512 + 512], o1[:, csl], AF.Square
                    )
                    mm(sts1[c][0:1, :], sum_lhs, o1[:, csl],
                       start=(ot == 0), stop=(ot == 3))
                    mm(sts1[c][64:65, :], sum_lhs,
                       sq1t[:, c * 512 : c * 512 + 512],
                       start=(ot == 0), stop=(ot == 3))

            # ---- phase 5: LN1 -> out ------------------------------------------
            otout = ap.tile([128, 4 * NQ], BF16, name="otout")
            rr1, rm1 = ln_stats_and_reps(sts1, "l1")
            for t in range(4):
                for c in range(2):
                    sl = slice(t * NQ + c * 512, t * NQ + c * 512 + 512)
                    rsl = slice(c * 512, c * 512 + 512)
                    tmp = sm.tile([128, 512], BF16, name=f"tmp{t}{c}", tag="rl",
                                  bufs=2)
                    nc.vector.tensor_mul(tmp[:], o1[:, sl], rr1[:, rsl])
                    if affine:
                        nc.vector.tensor_sub(tmp[:], tmp[:], rm1[:, rsl])
                        nc.vector.tensor_scalar(
                            otout[:, sl], tmp[:],
                            gb_sb[:, 8 + t : 8 + t + 1],
                            gb_sb[:, 12 + t : 12 + t + 1],
                            mult, add,
                        )
                    else:
                        nc.vector.tensor_sub(otout[:, sl], tmp[:], rm1[:, rsl])
                    nc.sync.dma_start(
                        out_d[t * 128 : (t + 1) * 128, c * 512 : c * 512 + 512],
                        otout[:, sl],
                    )

    _split_multi_waits(nc)
    return nc


_nc_cache = {}


def _get_nc(kt_tiles=5, affine=False):
    key = (kt_tiles, affine)
    if key not in _nc_cache:
        _nc_cache[key] = build_nc(kt_tiles, affine)
    return _nc_cache[key]


def _kt_tiles_for(mask):
    n = int(max(int((mask[b] != 0).sum()) for b in range(mask.shape[0])))
    return max(1, (n + 127) // 128)


def _is_affine(g0, b0, g1, b1):
    return not (
        np.all(np.asarray(g0) == 1.0)
        and np.all(np.asarray(b0) == 0.0)
        and np.all(np.asarray(g1) == 1.0)
        and np.all(np.asarray(b1) == 0.0)
    )


def prep_inputs(Q, K, mask, Wq, bq, Wk, bk, Wv, bv, Wo, bo, g0, b0, g1, b1,
                kt_tiles=None):
    f32, bf = np.float32, ml_dtypes.bfloat16
    if kt_tiles is None:
        kt_tiles = _kt_tiles_for(np.asarray(mask))
    nkp = kt_tiles * 128

    def percol(v):  # [512] feature vector -> [128, 4] per-partition layout
        return np.ascontiguousarray(np.asarray(v, f32).reshape(4, 128).T)

    wv_h = np.ascontiguousarray(
        np.vstack([np.asarray(Wv, f32), np.asarray(bv, f32)[None, :]])
    ).astype(bf)
    cr = np.zeros((2, 128), f32)
    cr[0, :] = 1.0
    cr = cr.astype(bf)
    gb = np.concatenate(
        [percol(g0), percol(b0), percol(g1), percol(b1)], axis=1
    ).astype(f32)
    cn = np.full((128, 1), 1.0 / D, f32).astype(bf)
    wq_h = np.ascontiguousarray(np.asarray(Wq, f32)).astype(bf)
    wk_h = np.ascontiguousarray(np.asarray(Wk, f32)).astype(bf)
    wo_h = np.ascontiguousarray(np.asarray(Wo, f32)).astype(bf)

    in_maps = []
    for b in range(B):
        qt = np.ascontiguousarray(np.asarray(Q[b], f32).T).astype(bf)
        idx = np.nonzero(np.asarray(mask)[b] != 0)[0]
        kc = np.zeros((nkp, D), f32)
        kc[: len(idx)] = np.asarray(K[b], f32)[idx]
        indrow = np.zeros((1, nkp), f32)
        indrow[0, : len(idx)] = 1.0
        kt = np.ascontiguousarray(np.vstack([kc.T, indrow])).astype(bf)
        ind = np.ascontiguousarray(indrow.reshape(kt_tiles, 128).T).astype(bf)
        in_maps.append(
            {
                "qt": qt,
                "kt": kt,
                "wq": wq_h,
                "wk": wk_h,
                "wv": wv_h,
                "wo": wo_h,
                "bq": percol(bq),
                "bk": percol(bk),
                "bo": percol(bo),
                "ind": ind,
                "cr": cr,
                "gb": gb,
                "cn": cn,
            }
        )
    return in_maps


def kernel(Q, K, mask, Wq, bq, Wk, bk, Wv, bv, Wo, bo, g0, b0, g1, b1):
    mask = np.asarray(mask)
    kt_tiles = _kt_tiles_for(mask)
    affine = _is_affine(g0, b0, g1, b1)
    nc = _get_nc(kt_tiles, affine)
    in_maps = prep_inputs(
        Q, K, mask, Wq, bq, Wk, bk, Wv, bv, Wo, bo, g0, b0, g1, b1, kt_tiles
    )
    res = run_bass_kernel_spmd(nc, in_maps, list(range(N_CORES)))
    out = np.stack(
        [np.ascontiguousarray(res.results[i]["out"].T) for i in range(N_CORES)]
    )
    return out.astype(np.float32)


# revision 46
# speedup vs baseline: 1.0391x; 1.0316x over previous
"""Trainium2 Bass kernel for the masked-attention block (nn_MAB_61607010894006).

Sharding: data-parallel over batch B=8 across 8 NeuronCores (one batch row
per core, weights replicated, no collectives).

v2 design (vs the 249us v1 baseline):
  - bf16 activations+weights (fp32 PSUM accumulation, fp32 LN stats, fp32
    output). Halves DMA, enables FWL on LDWEIGHTS and DVE 2x perf modes.
  - Scores: two 512-wide matmuls per (head, k-tile) into 1-bank PSUM
    tiles; head PAIRS run concurrently on the PE via row tiling
    (contraction=64: heads 2t/2t+1 at array rows 0-63/64-127).
  - Masking without exp-bias: masked k tokens compacted out on the host;
    padded k columns are zero (scores=0, exp=1) and both the v rows and
    the denominator indicator row are 0 there, so pads contribute nothing
    to numerator or denominator.
  - Softmax denominator rides the po matmul as a 65th v row (indicator).
    ALL reciprocals/rsqrts are computed as exp(-ln(x)) / exp(-0.5 ln(x))
    on ScalarE: Ln and Exp share one ACT table set, so the kernel does a
    single ACT_TABLE_LOAD (v1 lost 26us to exp<->recip table thrash).
    (Custom DVE ops and gpsimd compute ops fail codegen in this
    toolchain - only gpsimd DMA works.)
  - Row->partitions broadcasts via tiny PE ones-matmuls; elementwise on
    DVE in bf16 (2x mode); odd-head partition shifts via gpsimd DMA.
"""

import sys

sys.path.insert(0, "/opt/trn_rl_repo")

import numpy as np
import ml_dtypes

import concourse.bass as bass
import concourse.mybir as mybir
import concourse.tile as tile
from concourse.bass_utils import run_bass_kernel_spmd


F32 = mybir.dt.float32
F32R = mybir.dt.float32r
BF16 = mybir.dt.bfloat16
AF = mybir.ActivationFunctionType

B, NQ, NK, D, H, DH = 8, 1024, 1024, 512, 8, 64
EPS = 1e-5
N_CORES = 8


def _split_multi_waits(nc):
    """This toolchain's walrus allows ONE sem wait per TPB instruction; Tile
    can emit several (kernel-tail drain). Hoist extras onto preceding
    single-wait NOPs on the same engine stream (equivalent: in-order issue).
    """
    multi_update = []
    for fn in nc.m.functions:
        for bb in fn.blocks:
            insts = bb.instructions
            new = []
            changed = False
            for inst in insts:
                si = inst.sync_info
                if si is not None and si.on_wait and len(si.on_wait) > 1:
                    waits = list(si.on_wait)
                    for w in waits[:-1]:
                        nop = mybir.InstNoOp(
                            name=f"I-wsplit-{nc.next_id()}", engine=inst.engine
                        )
                        nop.sync_info = mybir.SyncInfo(on_wait=[w], on_update=[])
                        new.append(nop)
                    inst.sync_info = mybir.SyncInfo(
                        on_wait=[waits[-1]], on_update=list(si.on_update)
                    )
                    changed = True
                if si is not None and si.on_update and len(si.on_update) > 1:
                    multi_update.append(inst.name)
                new.append(inst)
            if changed:
                bb.instructions = new
    if multi_update:
        raise RuntimeError(f">1 sem update unsupported: {multi_update[:10]}")


def build_nc(kt_tiles=5, affine=False):
    NKP = kt_tiles * 128  # compacted+padded key/value token count
    nc = bass.Bass()

    qt_d = nc.dram_tensor("qt", [D, NQ], BF16, kind="ExternalInput")
    kt_d = nc.dram_tensor("kt", [D + 1, NKP], BF16, kind="ExternalInput")  # +ind
    wq_d = nc.dram_tensor("wq", [D, D], BF16, kind="ExternalInput")
    wk_d = nc.dram_tensor("wk", [D, D], BF16, kind="ExternalInput")
    wv_d = nc.dram_tensor("wv", [D + 1, D], BF16, kind="ExternalInput")  # +bv row
    wo_d = nc.dram_tensor("wo", [D, D], BF16, kind="ExternalInput")
    bq_d = nc.dram_tensor("bq", [128, 4], F32, kind="ExternalInput")
    bk_d = nc.dram_tensor("bk", [128, 4], F32, kind="ExternalInput")
    bo_d = nc.dram_tensor("bo", [128, 4], F32, kind="ExternalInput")
    ind_d = nc.dram_tensor("ind", [128, kt_tiles], BF16, kind="ExternalInput")
    cr_d = nc.dram_tensor("cr", [2, 128], BF16, kind="ExternalInput")  # row0=ones
    gb_d = nc.dram_tensor("gb", [128, 16], F32, kind="ExternalInput")  # percol
    cn_d = nc.dram_tensor("cn", [128, 1], BF16, kind="ExternalInput")  # 1/512
    out_d = nc.dram_tensor("out", [D, NQ], BF16, kind="ExternalOutput")

    mult, add = mybir.AluOpType.mult, mybir.AluOpType.add

    def mm(out, lhsT, rhs, **kw):
        nc.tensor.matmul(out, lhsT, rhs, **kw)

    with tile.TileContext(nc) as tc:
        with (
            tc.tile_pool(name="wp", bufs=1) as wp,
            tc.tile_pool(name="ap", bufs=1) as ap,
            tc.tile_pool(name="sm", bufs=2) as sm,
            tc.tile_pool(name="pp", bufs=1, space="PSUM") as pp,
        ):
            def ps_tile(shape, name, tag, bufs):
                return pp.tile(shape, F32, name=name, tag=tag, bufs=bufs)

            # ---- small constants first (cheap DMAs) ---------------------------
            bq_sb = wp.tile([128, 4], F32, name="bq_sb")
            bk_sb = wp.tile([128, 4], F32, name="bk_sb")
            bo_sb = wp.tile([128, 4], F32, name="bo_sb")
            ind_sb = wp.tile([128, kt_tiles], BF16, name="ind_sb")
            cr_sb = wp.tile([2, 128], BF16, name="cr_sb")
            gb_sb = wp.tile([128, 16], F32, name="gb_sb")
            cn_sb = wp.tile([128, 1], BF16, name="cn_sb")
            nc.sync.dma_start(bq_sb[:], bq_d[:])
            nc.sync.dma_start(bk_sb[:], bk_d[:])
            nc.sync.dma_start(bo_sb[:], bo_d[:])
            nc.sync.dma_start(ind_sb[:], ind_d[:])
            nc.sync.dma_start(cr_sb[:], cr_d[:])
            nc.sync.dma_start(gb_sb[:], gb_d[:])
            nc.sync.dma_start(cn_sb[:], cn_d[:])
            eps_sb = wp.tile([1, 1], F32, name="eps_sb")
            nc.vector.memset(eps_sb[:], EPS)
            sum_lhs = cn_sb[:, 0:1]                    # [128,1] bf16 = 1/512
            ones_row = cr_sb[0:1, :]                   # [1,128] bf16 lhsT

            # ---- PE warmup while DMAs stream ---------------------------------
            wu_sb = wp.tile([128, 128], BF16, name="wu_sb")
            nc.vector.memset(wu_sb[:], 0.001)
            wu_ps = ps_tile([128, 512], "wu_ps", "s1", 2)
            for i in range(44):
                mm(wu_ps[:, 0:128], wu_sb[:], wu_sb[:],
                   start=(i == 0), stop=(i == 43))
            wu_out = wp.tile([1, 1], F32, name="wu_out")
            nc.vector.tensor_copy(wu_out[:], wu_ps[0:1, 0:1])

            # ---- weights ------------------------------------------------------
            wq_sb = wp.tile([128, 4 * D], BF16, name="wq_sb")
            wk_sb = wp.tile([128, 4 * D], BF16, name="wk_sb")
            wv_sb = wp.tile([128, 4 * D], BF16, name="wv_sb")
            wv1_sb = wp.tile([1, D], BF16, name="wv1_sb")
            wo_sb = wp.tile([128, 4 * D], BF16, name="wo_sb")
            for t in range(4):
                nc.scalar.dma_start(
                    wv_sb[:, t * D : (t + 1) * D], wv_d[t * 128 : (t + 1) * 128, :]
                )
            nc.scalar.dma_start(wv1_sb[:, :], wv_d[D : D + 1, :])

            # ---- staged inputs (kt first: v+k projections start earliest) ----
            kt_sb = wp.tile([128, 4 * NKP], BF16, name="kt_sb")
            kt1_sb = wp.tile([1, NKP], BF16, name="kt1_sb")
            for t in range(4):
                nc.sync.dma_start(
                    kt_sb[:, t * NKP : (t + 1) * NKP],
                    kt_d[t * 128 : (t + 1) * 128, :],
                )
            nc.sync.dma_start(kt1_sb[:, :], kt_d[D : D + 1, :])
            for t in range(4):
                nc.scalar.dma_start(
                    wk_sb[:, t * D : (t + 1) * D], wk_d[t * 128 : (t + 1) * 128, :]
                )
            qt_sb = wp.tile([128, 4 * NQ], BF16, name="qt_sb")
            for t in range(4):
                nc.sync.dma_start(
                    qt_sb[:, t * NQ : (t + 1) * NQ],
                    qt_d[t * 128 : (t + 1) * 128, :],
                )
            for t in range(4):
                nc.scalar.dma_start(
                    wq_sb[:, t * D : (t + 1) * D], wq_d[t * 128 : (t + 1) * 128, :]
                )
            for t in range(4):
                nc.sync.dma_start(
                    wo_sb[:, t * D : (t + 1) * D], wo_d[t * 128 : (t + 1) * 128, :]
                )

            # ---- persistent activations --------------------------------------
            q_bf = ap.tile([128, 4 * NQ], BF16, name="q_bf")
            k_bf = ap.tile([128, 4 * NKP], BF16, name="k_bf")
            v_sb = ap.tile([128, kt_tiles * (8 * 65)], BF16, name="v_sb")
            v_ones = v_sb.rearrange("p (v h x) -> p v h x", v=kt_tiles, h=8)[
                :, :, :, 64
            ]
            nc.vector.tensor_copy(
                v_ones,
                ind_sb.rearrange("p (v a) -> p v a", a=1)
                .broadcast_to([128, kt_tiles, 8]),
            )

            # ---- phase 1a: v projection (token-major, +bias via ind row) -----
            for vt in range(kt_tiles):
                ps_v = ps_tile([128, 512], "ps_v", "s1", 2)
                for kc in range(4):
                    mm(
                        ps_v[:],
                        kt_sb[:, kc * NKP + vt * 128 : kc * NKP + (vt + 1) * 128],
                        wv_sb[:, kc * D : (kc + 1) * D],
                        start=(kc == 0),
                        stop=False,
                    )
                mm(
                    ps_v[:],
                    kt1_sb[0:1, vt * 128 : (vt + 1) * 128],
                    wv1_sb[0:1, :],
                    start=False,
                    stop=True,
                )
                v_dst = v_sb[:, vt * 520 : (vt + 1) * 520].rearrange(
                    "p (h x) -> p h x", h=8
                )[:, :, 0:64]
                nc.scalar.copy(v_dst, ps_v.rearrange("p (h x) -> p h x", h=8))

            # ---- phases 1b+2 interleaved per t-block -------------------------
            o_bf = ap.tile([128, 4 * NQ], BF16, name="o_bf")
            sqt_tiles = []
            kchunks = [(0, 512), (512, NKP - 512)] if NKP > 512 else [(0, NKP)]

            def keepwarm(n, name):
                kw = ps_tile([128, 512], name, "po1", 3)
                for i in range(n):
                    mm(kw[:], wu_sb[:], wq_sb[:, 0:512],
                       start=(i == 0), stop=(i == n - 1))

            # ---- layernorm sums + stats + broadcast helpers -------------------
            def ln_sums_make(tag):
                return [ps_tile([65, 512], f"st{tag}{c}", "sx", 3)
                        for c in range(2)]

            def ln_sums_step(sts, x_bf, sq_tiles, ts):
                for c in range(2):
                    for t in ts:
                        csl = slice(t * NQ + c * 512, t * NQ + c * 512 + 512)
                        mm(sts[c][0:1, :], sum_lhs, x_bf[:, csl],
                           start=(t == 0), stop=(t == 3))
                        mm(sts[c][64:65, :], sum_lhs,
                           sq_tiles[t][:, c * 512 : c * 512 + 512],
                           start=(t == 0), stop=(t == 3))

            def ln_sums(x_bf, sq_tiles, tag):
                sts = ln_sums_make(tag)
                ln_sums_step(sts, x_bf, sq_tiles, range(4))
                return sts

            def ln_stats_and_reps(sts, tag):
                """sts: per-c [65,512] f32 PSUM (mean row 0, meansq row 64).
                Returns (rr, rm): [128, NQ] bf16 broadcasts of rstd, mu*rstd."""
                var = sm.tile([1, NQ], F32, name=f"var{tag}", tag="var", bufs=2)
                for c in range(2):
                    csl = slice(c * 512, (c + 1) * 512)
                    nc.scalar.activation(var[:, csl], sts[c][0:1, :], AF.Square)
                    nc.vector.tensor_sub(var[:, csl], sts[c][64:65, :],
                                         var[:, csl])
                # rstd = exp(-0.5*ln(var+eps)) — stays in the Exp table set;
                # per-c chains so chunk 0's broadcasts start before chunk 1
                rstd = sm.tile([1, NQ], BF16, name=f"rs{tag}", tag="rs", bufs=2)
                murm = sm.tile([1, NQ], BF16, name=f"mm{tag}", tag="mm2", bufs=2)
                rr = sm.tile([128, NQ], BF16, name=f"rrb{tag}", tag="rrb", bufs=2)
                rm = sm.tile([128, NQ], BF16, name=f"rmb{tag}", tag="rmb", bufs=2)
                for c in range(2):
                    csl = slice(c * 512, (c + 1) * 512)
                    nc.scalar.activation(var[:, csl], var[:, csl], AF.Ln,
                                         bias=eps_sb[0:1, 0:1])
                    nc.scalar.activation(rstd[:, csl], var[:, csl], AF.Exp,
                                         scale=-0.5)
                    nc.vector.tensor_mul(murm[:, csl], sts[c][0:1, :],
                                         rstd[:, csl])
                    rr_ps = ps_tile([128, 512], f"rr{tag}{c}", "sx", 3)
                    mm(rr_ps[:], ones_row, rstd[:, csl], start=True, stop=True)
                    nc.vector.tensor_copy(rr[:, csl], rr_ps[:])
                    rm_ps = ps_tile([128, 512], f"rm{tag}{c}", "sx", 3)
                    mm(rm_ps[:], ones_row, murm[:, csl], start=True, stop=True)
                    nc.vector.tensor_copy(rm[:, csl], rm_ps[:])
                return rr, rm


            for t in range(4):
                tsl = slice(t * NQ, (t + 1) * NQ)
                # -- k projection block t (kc-outer, weights reused) --
                ps_k = [
                    ps_tile([128, cw], f"ps_k{t}_{ci}", "s1", 2)
                    for ci, (cs, cw) in enumerate(kchunks)
                ]
                for kc in range(4):
                    for ci, (cs, cw) in enumerate(kchunks):
                        mm(
                            ps_k[ci][:],
                            wk_sb[:, kc * D + t * 128 : kc * D + (t + 1) * 128],
                            kt_sb[:, kc * NKP + cs : kc * NKP + cs + cw],
                            start=(kc == 0),
                            stop=(kc == 3),
                        )
                for ci, (cs, cw) in enumerate(kchunks):
                    nc.vector.tensor_scalar_add(
                        k_bf[:, t * NKP + cs : t * NKP + cs + cw],
                        ps_k[ci][:],
                        bk_sb[:, t : t + 1],
                    )
                # -- q projection block t --
                ps_q = [ps_tile([128, 512], f"ps_q{t}_{c}", "s1", 2) for c in range(2)]
                for kc in range(4):
                    for c in range(2):
                        mm(
                            ps_q[c][:],
                            wq_sb[:, kc * D + t * 128 : kc * D + (t + 1) * 128],
                            qt_sb[:, kc * NQ + c * 512 : kc * NQ + c * 512 + 512],
                            start=(kc == 0),
                            stop=(kc == 3),
                        )
                for c in range(2):
                    nc.vector.tensor_scalar_add(
                        q_bf[:, t * NQ + c * 512 : t * NQ + c * 512 + 512],
                        ps_q[c][:],
                        bq_sb[:, t : t + 1],
                    )

                # -- scores + exp for head pair (2t, 2t+1), row-tiled --
                at_tiles = {0: [], 1: []}
                for i in range(kt_tiles):
                    for sub in range(2):
                        rh = sub * 64
                        at_sb = ap.tile(
                            [128, NQ], BF16, name=f"at{t}_{i}_{sub}", tag="at",
                            bufs=16,
                        )
                        at_tiles[sub].append(at_sb)
                        for c in range(2):
                            ps_s = ps_tile([128, 512], f"s{t}_{i}_{sub}_{c}", "sx", 3)
                            mm(
                                ps_s[:],
                                k_bf[rh : rh + 64,
                                     t * NKP + i * 128 : t * NKP + (i + 1) * 128],
                                q_bf[rh : rh + 64,
                                     t * NQ + c * 512 : t * NQ + c * 512 + 512],
                                start=True,
                                stop=True,
                            )
                            nc.scalar.activation(
                                at_sb[:, c * 512 : (c + 1) * 512], ps_s[:],
                                AF.Exp, scale=0.125,
                            )

                # -- early LN0 sums (t<=2 blocks) during the last pair --
                if t == 3:
                    sts0 = ln_sums_make("l0")
                    ln_sums_step(sts0, o_bf, sqt_tiles, range(3))

                # -- A@V (both chunks), then normalize + residual --
                for sub in range(2):
                    h = 2 * t + sub
                    rh = sub * 64
                    pos = []
                    for c in range(2):
                        po = ps_tile([65, 512], f"po{h}_{c}", "po1", 3)
                        pos.append(po)
                        for i in range(kt_tiles):
                            mm(
                                po[:],
                                v_sb[:, i * 520 + h * 65 : i * 520 + (h + 1) * 65],
                                at_tiles[sub][i][:, c * 512 : (c + 1) * 512],
                                start=(i == 0),
                                stop=(i == kt_tiles - 1),
                            )
                    for c in range(2):
                        po = pos[c]
                        lnr = sm.tile([1, 512], F32, name=f"ln{h}{c}",
                                      tag="ri", bufs=2)
                        nc.scalar.activation(lnr[:], po[64:65, :], AF.Ln)
                        rinv = sm.tile([1, 512], BF16, name=f"r{h}{c}",
                                       tag="ri2", bufs=2)
                        nc.scalar.activation(rinv[:], lnr[:], AF.Exp,
                                             scale=-1.0)
                        pb = ps_tile([64, 512], f"pb{h}{c}", "po1", 3)
                        mm(pb[:], cr_sb[0:1, 0:64], rinv[:],
                           start=True, stop=True)
                        rb = sm.tile([64, 512], BF16, name=f"rb{h}{c}",
                                     tag="rb", bufs=2)
                        nc.vector.tensor_copy(rb[:], pb[:])
                        avn = sm.tile([64, 512], BF16, name=f"av{h}{c}",
                                      tag="av", bufs=2)
                        nc.vector.tensor_mul(avn[:], po[0:64, :], rb[:])
                        csl = slice(t * NQ + c * 512, t * NQ + c * 512 + 512)
                        if rh == 0:
                            nc.vector.tensor_add(
                                o_bf[0:64, csl], avn[:], q_bf[0:64, csl]
                            )
                        else:
                            av2 = sm.tile([128, 512], BF16, name=f"av2_{h}{c}",
                                          tag="av2", bufs=2)
                            nc.sync.dma_start(av2[64:128, :], avn[:])
                            nc.vector.tensor_add(
                                o_bf[64:128, csl], av2[64:128, :],
                                q_bf[64:128, csl],
                            )

                # square of o block t for LN0 sumsq (sums deferred)
                sqt = sm.tile([128, NQ], BF16, name=f"sqt{t}", tag="sqt", bufs=4)
                sqt_tiles.append(sqt)
                nc.vector.tensor_mul(sqt[:], o_bf[:, tsl], o_bf[:, tsl])

            # ---- phase 3: LN0 -------------------------------------------------
            ot0 = ap.tile([128, 4 * NQ], BF16, name="ot0")
            ln_sums_step(sts0, o_bf, sqt_tiles, [3])
            rr0, rm0 = ln_stats_and_reps(sts0, "l0")
            for c in range(2):
                for t in range(4):
                    sl = slice(t * NQ + c * 512, t * NQ + c * 512 + 512)
                    rsl = slice(c * 512, c * 512 + 512)
                    nc.vector.tensor_mul(ot0[:, sl], o_bf[:, sl], rr0[:, rsl])
                    nc.vector.tensor_sub(ot0[:, sl], ot0[:, sl], rm0[:, rsl])
                    if affine:
                        nc.vector.tensor_scalar(
                            ot0[:, sl], ot0[:, sl],
                            gb_sb[:, 0 + t : 0 + t + 1],
                            gb_sb[:, 4 + t : 4 + t + 1],
                            mult, add,
                        )

            # ---- phase 4: FC + relu + residual, token-chunk split ------------
            o1 = ap.tile([128, 4 * NQ], BF16, name="o1")
            sq1_tiles = [
                sm.tile([128, NQ], BF16, name=f"sq1t{ot}", tag="sqt", bufs=4)
                for ot in range(4)
            ]
            sts1 = ln_sums_make("l1")
            for c in range(2):
                for ot in range(4):
                    ps_f = ps_tile([128, 512], f"psf{ot}_{c}", "s1", 2)
                    for ft in range(4):
                        mm(
                            ps_f[:],
                            wo_sb[:, ft * D + ot * 128 : ft * D + (ot + 1) * 128],
                            ot0[:, ft * NQ + c * 512 : ft * NQ + c * 512 + 512],
                            start=(ft == 0),
                            stop=(ft == 3),
                        )
                    csl = slice(ot * NQ + c * 512, ot * NQ + c * 512 + 512)
                    rl = sm.tile([128, 512], BF16, name=f"rl{ot}{c}", tag="rl",
                                 bufs=2)
                    nc.scalar.activation(
                        rl[:], ps_f[:], AF.Relu, bias=bo_sb[:, ot : ot + 1]
                    )
                    nc.vector.tensor_add(o1[:, csl], ot0[:, csl], rl[:])
                    sq1t = sq1_tiles[ot]
                    nc.scalar.activation(
                        sq1t[:, c * 512 : c * 512 + 512], o1[:, csl], AF.Square
                    )
                    mm(sts1[c][0:1, :], sum_lhs, o1[:, csl],
                       start=(ot == 0), stop=(ot == 3))
                    mm(sts1[c][64:65, :], sum_lhs,
                       sq1t[:, c * 512 : c * 512 + 512],
                       start=(ot == 0), stop=(ot == 3))

            # ---- phase 5: LN1 -> out ------------------------------------------
            otout = ap.tile([128, 4 * NQ], BF16, name="otout")
            rr1, rm1 = ln_stats_and_reps(sts1, "l1")
            for t in range(4):
                for c in range(2):
                    sl = slice(t * NQ + c * 512, t * NQ + c * 512 + 512)
                    rsl = slice(c * 512, c * 512 + 512)
                    tmp = sm.tile([128, 512], BF16, name=f"tmp{t}{c}", tag="rl",
                                  bufs=2)
                    nc.vector.tensor_mul(tmp[:], o1[:, sl], rr1[:, rsl])
                    if affine:
                        nc.vector.tensor_sub(tmp[:], tmp[:], rm1[:, rsl])
                        nc.vector.tensor_scalar(
                            otout[:, sl], tmp[:],
                            gb_sb[:, 8 + t : 8 + t + 1],
                            gb_sb[:, 12 + t : 12 + t + 1],
                            mult, add,
                        )
                    else:
                        nc.vector.tensor_sub(otout[:, sl], tmp[:], rm1[:, rsl])
                    nc.sync.dma_start(
                        out_d[t * 128 : (t + 1) * 128, c * 512 : c * 512 + 512],
                        otout[:, sl],
                    )

    _split_multi_waits(nc)
    return nc


_nc_cache = {}


def _get_nc(kt_tiles=5, affine=False):
    key = (kt_tiles, affine)
    if key not in _nc_cache:
        _nc_cache[key] = build_nc(kt_tiles, affine)
    return _nc_cache[key]


def _kt_tiles_for(mask):
    n = int(max(int((mask[b] != 0).sum()) for b in range(mask.shape[0])))
    return max(1, (n + 127) // 128)


def _is_affine(g0, b0, g1, b1):
    return not (
        np.all(np.asarray(g0) == 1.0)
        and np.all(np.asarray(b0) == 0.0)
        and np.all(np.asarray(g1) == 1.0)
        and np.all(np.asarray(b1) == 0.0)
    )


def prep_inputs(Q, K, mask, Wq, bq, Wk, bk, Wv, bv, Wo, bo, g0, b0, g1, b1,
                kt_tiles=None):
    f32, bf = np.float32, ml_dtypes.bfloat16
    if kt_tiles is None:
        kt_tiles = _kt_tiles_for(np.asarray(mask))
    nkp = kt_tiles * 128

    def percol(v):  # [512] feature vector -> [128, 4] per-partition layout
        return np.ascontiguousarray(np.asarray(v, f32).reshape(4, 128).T)

    wv_h = np.ascontiguousarray(
        np.vstack([np.asarray(Wv, f32), np.asarray(bv, f32)[None, :]])
    ).astype(bf)
    cr = np.zeros((2, 128), f32)
    cr[0, :] = 1.0
    cr = cr.astype(bf)
    gb = np.concatenate(
        [percol(g0), percol(b0), percol(g1), percol(b1)], axis=1
    ).astype(f32)
    cn = np.full((128, 1), 1.0 / D, f32).astype(bf)
    wq_h = np.ascontiguousarray(np.asarray(Wq, f32)).astype(bf)
    wk_h = np.ascontiguousarray(np.asarray(Wk, f32)).astype(bf)
    wo_h = np.ascontiguousarray(np.asarray(Wo, f32)).astype(bf)

    in_maps = []
    for b in range(B):
        qt = np.ascontiguousarray(np.asarray(Q[b], f32).T).astype(bf)
        idx = np.nonzero(np.asarray(mask)[b] != 0)[0]
        kc = np.zeros((nkp, D), f32)
        kc[: len(idx)] = np.asarray(K[b], f32)[idx]
        indrow = np.zeros((1, nkp), f32)
        indrow[0, : len(idx)] = 1.0
        kt = np.ascontiguousarray(np.vstack([kc.T, indrow])).astype(bf)
        ind = np.ascontiguousarray(indrow.reshape(kt_tiles, 128).T).astype(bf)
        in_maps.append(
            {
                "qt": qt,
                "kt": kt,
                "wq": wq_h,
                "wk": wk_h,
                "wv": wv_h,
                "wo": wo_h,
                "bq": percol(bq),
                "bk": percol(bk),
                "bo": percol(bo),
                "ind": ind,
                "cr": cr,
                "gb": gb,
                "cn": cn,
            }
        )
    return in_maps


def kernel(Q, K, mask, Wq, bq, Wk, bk, Wv, bv, Wo, bo, g0, b0, g1, b1):
    mask = np.asarray(mask)
    kt_tiles = _kt_tiles_for(mask)
    affine = _is_affine(g0, b0, g1, b1)
    nc = _get_nc(kt_tiles, affine)
    in_maps = prep_inputs(
        Q, K, mask, Wq, bq, Wk, bk, Wv, bv, Wo, bo, g0, b0, g1, b1, kt_tiles
    )
    res = run_bass_kernel_spmd(nc, in_maps, list(range(N_CORES)))
    out = np.stack(
        [np.ascontiguousarray(res.results[i]["out"].T) for i in range(N_CORES)]
    )
    return out.astype(np.float32)
